# revision 1
# baseline (speedup 1.0000x reference)
"""Trainium2 Bass kernel for 16-head causal attention (transposed-softmax variant).

Problem shapes: x [8, 1024, 1024]; W_K/W_Q/W_V [16, 64, 1024]; W_O [1024, 1024].
Sharding: pure data-parallel over batch (8 batch elements -> 8 cores), weights
replicated, no collectives.

Per-core pipeline (one batch element, seq=1024, d_embed=1024, 16 heads x 64):
  1. QKV projections as K_T/Q_T [heads*64, seq] and V [seq, heads*64], fp16
     operands, fp32 PSUM accumulation. W_Q is pre-scaled by 1/sqrt(d_head) on
     the host so scores come out pre-scaled.
  2. Per head pair: scores S[c, C] = K[c].Q[C] for causal-allowed C-chunks
     only, the two heads' K=64 matmuls interleaved so they run concurrently in
     disjoint PE row-groups; the triangular mask of the diagonal 128-block is
     accumulated into PSUM via an identity-matmul (N=128).
  3. Softmax over C without max-subtraction (scores ~ N(0,1), exp cannot
     overflow): one E=exp(S) per head-row on the scalar engine, accum_out
     giving the row sum; normalization is folded into V rows (V' = V/rowsum).
     E rows are stored compactly (only the causally-valid prefix).
  4. Z^T[h, C] += V'^T E per c-tile (descending i, N trimmed to the causal
     prefix, two heads col-split in one PSUM bank); output projection
     Z_flat @ W_O^T.

Scheduling: attention is scalar-engine-heavy while the projections are
scalar-idle, and score-row PSUM slots gate each row on the previous row's exp.
So dense projection groups are interleaved as fillers into the attention rows
of pairs 0-3, and each pair's AV block is interleaved into the next pair's
rows for pairs 4-7 — the PE never waits on the exp latency chain.
"""

import numpy as np

S, E, A, H, B = 1024, 1024, 16, 64, 8
P = 128          # partitions
NEG = -30000.0   # additive mask value (fp16-safe; exp -> 0 in fp32)

_cache = {}


def _off(i):
    """Compact E-buffer offset of row-tile i (valid width of row i is (i+1)*P)."""
    return P * i * (i + 1) // 2


EW = _off(8)     # 4608 columns total


def _build_nc():
    import concourse.bass as bass
    import concourse.mybir as mybir
    from concourse.tile import TileContext

    f16 = mybir.dt.float16
    f32 = mybir.dt.float32
    Exp = mybir.ActivationFunctionType.Exp

    nc = bass.Bass()
    xt_d = nc.dram_tensor("xt", [E, S], f16, kind="ExternalInput")        # x[b].T
    wkqv_d = nc.dram_tensor("wkqv", [E, 3 * A * H], f16, kind="ExternalInput")
    wo_d = nc.dram_tensor("wo", [A * H, E], f16, kind="ExternalInput")    # W_O.T
    msk_d = nc.dram_tensor("msk", [P, P], f16, kind="ExternalInput")
    id_d = nc.dram_tensor("ident", [P, P], f16, kind="ExternalInput")
    out_d = nc.dram_tensor("out", [S, E], f16, kind="ExternalOutput")

    ET = E // P       # 8 e-tiles
    CT = S // P       # 8 c-tiles
    NC = S // 512     # 2 512-chunks

    with TileContext(nc) as tc:
        with (
            tc.tile_pool(name="inp", bufs=1) as inp,
            tc.tile_pool(name="kqv", bufs=1) as kqv,
            tc.tile_pool(name="epool", bufs=4) as epool,
            tc.tile_pool(name="stats", bufs=12) as stats,
            tc.tile_pool(name="outp", bufs=3) as outp,
            tc.tile_pool(name="psq", bufs=4, space="PSUM") as psq,
            tc.tile_pool(name="pss", bufs=2, space="PSUM") as pss,
        ):
            # ---- SBUF destinations ----
            xT = inp.tile([P, ET, S], f16, tag="xT")
            wkqv = inp.tile([P, ET, 3 * A * H], f16, tag="wkqv")
            wo = inp.tile([P, ET, E], f16, tag="wo")
            msk = inp.tile([P, P], f16, tag="msk")
            ident = inp.tile([P, P], f16, tag="ident")

            # ---- loads, ordered so first-needed data lands first ----
            nc.sync.dma_start(ident[:], id_d[:])
            nc.sync.dma_start(msk[:], msk_d[:])
            nc.sync.dma_start(wkqv[:, 0, 0:512], wkqv_d[0:P, 0:512])
            nc.sync.dma_start(wkqv[:, 0, 512:A * H], wkqv_d[0:P, 512:A * H])
            nc.sync.dma_start(xT[:, 0, 0:256], xt_d[0:P, 0:256])
            nc.sync.dma_start(xT[:, 0, 256:512], xt_d[0:P, 256:512])
            for t in range(1, ET):  # K weights + first x half
                nc.sync.dma_start(wkqv[:, t, 0:A * H], wkqv_d[t * P:(t + 1) * P, 0:A * H])
                nc.sync.dma_start(xT[:, t, 0:512], xt_d[t * P:(t + 1) * P, 0:512])
            for t in range(ET):  # Q weights + second x half
                nc.sync.dma_start(wkqv[:, t, A * H:2 * A * H],
                                  wkqv_d[t * P:(t + 1) * P, A * H:2 * A * H])
                nc.sync.dma_start(xT[:, t, 512:S], xt_d[t * P:(t + 1) * P, 512:S])
            for t in range(ET):  # V weights
                nc.sync.dma_start(wkqv[:, t, 2 * A * H:3 * A * H],
                                  wkqv_d[t * P:(t + 1) * P, 2 * A * H:3 * A * H])
            for t in range(ET):  # output projection weights (needed last)
                nc.sync.dma_start(wo[:, t, :], wo_d[t * P:(t + 1) * P, :])

            K_T = kqv.tile([P, A // 2, S], f16, tag="K_T")   # pair-stacked [2h, c]
            Q_T = kqv.tile([P, A // 2, S], f16, tag="Q_T")
            V = kqv.tile([P, CT, A * H], f16, tag="V")       # [c, f]
            zT = kqv.tile([P, A // 2, S], f16, tag="zT")     # pair-stacked [f, C]

            # ---- PE warm-up: ride out the HAM throttle during the DMA wait ----
            wps = psq.tile([P, 512], f32, tag="psq", name="warm")
            for w in range(38):
                nc.tensor.matmul(wps[:, :P], ident[:], msk[:],
                                 start=(w == 0), stop=(w == 37),
                                 skip_group_check=True)
            wsb = stats.tile([P, 1], f32, tag="ssum", name="warmsink")
            nc.vector.reduce_max(wsb[:], wps[:, :P], axis=mybir.AxisListType.X)

            # ---- dense projection groups (also used as attention fillers) ----
            def kq_group(p, mat, cc):
                dst = K_T if mat == 0 else Q_T
                ps = psq.tile([P, 512], f32, tag="psq", name=f"q{p}{mat}{cc}")
                for et in range(ET):
                    nc.tensor.matmul(
                        ps[:],
                        wkqv[:, et, mat * A * H + p * P: mat * A * H + (p + 1) * P],
                        xT[:, et, cc * 512:(cc + 1) * 512],
                        start=(et == 0), stop=(et == ET - 1),
                    )
                nc.vector.tensor_copy(out=dst[:, p, cc * 512:(cc + 1) * 512], in_=ps[:])

            def v_group(fc, i):
                ps = psq.tile([P, 512], f32, tag="psq", name=f"v{fc}{i}")
                for et in range(ET):
                    nc.tensor.matmul(
                        ps[:],
                        xT[:, et, i * P:(i + 1) * P],
                        wkqv[:, et, 2 * A * H + fc * 512: 2 * A * H + (fc + 1) * 512],
                        start=(et == 0), stop=(et == ET - 1),
                    )
                nc.vector.tensor_copy(out=V[:, i, fc * 512:(fc + 1) * 512], in_=ps[:])

            def kq_pair_closures(p):
                return [lambda p=p, mat=mat, cc=cc: kq_group(p, mat, cc)
                        for cc in range(NC) for mat in (0, 1)]

            def v_closures(fc):
                return [lambda fc=fc, i=i: v_group(fc, i) for i in range(CT)]

            # ---- attention ----
            def attn_rows(p, fillers, front_load=False):
                """Scores+exp+V-scale rows of pair p, interleaving filler
                closures between rows. Returns (heads, E tiles)."""
                heads = [(2 * p, 0), (2 * p + 1, H)]
                Ets = [epool.tile([P, EW], f16, tag="E", name=f"E{k}_{p}")
                       for k in range(2)]
                fq = list(fillers)
                if front_load:   # short rows have the most exp-latency slack
                    w = [2, 2, 2, 2, 1, 1, 1, 1]
                    tot = sum(w)
                    share = [max(1, round(len(fq) * wi / tot)) if fq else 0 for wi in w]
                else:
                    share = [(len(fq) + CT - 1) // CT if fq else 0] * CT
                fi = 0
                for i in range(CT):
                    n_i = i // 4 + 1
                    vw = (i + 1) * P          # causally-valid row width
                    if i < 4:   # short rows fit a 512-wide psq slot; using the
                        # other pool splits the exp-release chain between rows
                        rows = [psq.tile([P, 512], f32, tag="psq", name=f"r{k}_{i}")
                                for k in range(2)]
                    else:
                        rows = [pss.tile([P, 1024], f32, tag="srow", name=f"r{k}_{i}")
                                for k in range(2)]
                    for j in range(n_i):
                        diag = j == n_i - 1
                        ntrim = (i * P - j * 512) + P if diag else 512
                        for k, (a, off) in enumerate(heads):
                            nc.tensor.matmul(
                                rows[k][:, j * 512:j * 512 + ntrim],
                                K_T[off:off + H, p, i * P:(i + 1) * P],
                                Q_T[off:off + H, p, j * 512:j * 512 + ntrim],
                                start=True, stop=not diag,
                                skip_group_check=True,
                            )
                        if diag:
                            for k in range(2):
                                nc.tensor.matmul(
                                    rows[k][:, i * P:(i + 1) * P],
                                    ident[:], msk[:],
                                    start=False, stop=True,
                                    skip_group_check=True,
                                )
                    for k, (a, off) in enumerate(heads):
                        sc = stats.tile([P, 1], f32, tag="ssum", name=f"sc{k}_{i}")
                        nc.scalar.activation(
                            Ets[k][:, _off(i):_off(i) + vw], rows[k][:, :vw], Exp,
                            accum_out=sc[:],
                        )
                        rcp = stats.tile([P, 1], f32, tag="rcp", name=f"rc{k}_{i}")
                        nc.vector.reciprocal(rcp[:], sc[:])
                        nc.vector.tensor_scalar_mul(
                            V[:, i, a * H:(a + 1) * H],
                            V[:, i, a * H:(a + 1) * H],
                            rcp[:],
                        )
                    for _ in range(share[i]):
                        if fi < len(fq):
                            fq[fi]()
                            fi += 1
                while fi < len(fq):
                    fq[fi]()
                    fi += 1
                return heads, Ets

            def av_closures(p, heads, Ets):
                """AV block of pair p as filler closures (descending i, causal
                N-trim, two heads col-split in one PSUM bank per chunk j)."""
                state = {}
                cs = []

                def step(j, i):
                    if i == CT - 1:
                        state[j] = psq.tile([P, 512], f32, tag="psq",
                                            name=f"za_{p}_{j}")
                    za = state[j]
                    ntrim = min(512, (i - 4 * j) * P + P)
                    for k, (a, off) in enumerate(heads):
                        nc.tensor.matmul(
                            za[off:off + H, :ntrim],
                            V[:, i, a * H:(a + 1) * H],
                            Ets[k][:, _off(i) + j * 512:_off(i) + j * 512 + ntrim],
                            start=(i == CT - 1), stop=(i == 4 * j),
                            skip_group_check=True,
                        )

                def copy(j):
                    nc.vector.tensor_copy(out=zT[:, p, j * 512:(j + 1) * 512],
                                          in_=state[j][:])

                for j in range(NC):
                    for i in range(CT - 1, 4 * j - 1, -1):
                        cs.append(lambda j=j, i=i: step(j, i))
                    cs.append(lambda j=j: copy(j))
                return cs

            # ---- merged schedule ----
            for pp, mat, cc in ((0, 0, 0), (1, 0, 0), (0, 1, 0), (1, 1, 0),
                                (0, 0, 1), (1, 0, 1), (0, 1, 1), (1, 1, 1)):
                kq_group(pp, mat, cc)
            for cl in v_closures(0):
                cl()

            pair_fillers = {
                0: kq_pair_closures(2) + kq_pair_closures(3),
                1: kq_pair_closures(4) + kq_pair_closures(5),
                2: kq_pair_closures(6) + kq_pair_closures(7),
                3: v_closures(1),
            }
            av_prev = None
            for p in range(8):
                fillers = pair_fillers.get(p, [])
                if av_prev is not None:
                    fillers = av_prev + fillers
                    av_prev = None
                heads, Ets = attn_rows(p, fillers, front_load=(p >= 4))
                avs = av_closures(p, heads, Ets)
                if p >= 3:
                    av_prev = avs          # interleave into next pair's rows
                else:
                    for cl in avs:
                        cl()
            for cl in av_prev:             # AV of pair 7
                cl()

            # ---- output projection ----
            for m in range(CT):
                for n_ in range(NC):
                    ps = psq.tile([P, 512], f32, tag="psq", name=f"o{m}{n_}")
                    for p2 in range(ET):
                        nc.tensor.matmul(
                            ps[:],
                            zT[:, p2, m * P:(m + 1) * P],
                            wo[:, p2, n_ * 512:(n_ + 1) * 512],
                            start=(p2 == 0), stop=(p2 == ET - 1),
                        )
                    ot = outp.tile([P, 512], f16, tag="ot")
                    last = m == CT - 1
                    if last:   # tail-critical: scalar-engine copy + quartered DMAs
                        nc.scalar.copy(out=ot[:], in_=ps[:])
                    else:
                        nc.vector.tensor_copy(out=ot[:], in_=ps[:])
                    nq = 4 if last else 2
                    w = 512 // nq
                    for hh in range(nq):
                        nc.sync.dma_start(
                            out_d[m * P:(m + 1) * P,
                                  n_ * 512 + hh * w:n_ * 512 + (hh + 1) * w],
                            ot[:, hh * w:(hh + 1) * w],
                        )

    # HW allows only one sync-wait per instruction (matmuls especially);
    # split excess waits into InstEventSemaphore like the bacc layer does.
    import bass_rust
    bass_rust.generate_event_semaphores(nc)
    return nc


def _host_prep(x, W_K, W_Q, W_V, W_O):
    """Pack per-core input dicts (host-side layout prep, fp16 casts)."""
    wk = W_K.transpose(2, 0, 1).reshape(E, A * H)
    wq = (W_Q / np.sqrt(H)).transpose(2, 0, 1).reshape(E, A * H)
    wv = W_V.transpose(2, 0, 1).reshape(E, A * H)
    wkqv = np.concatenate([wk, wq, wv], axis=1).astype(np.float16)
    wo = np.ascontiguousarray(W_O.T).astype(np.float16)

    r = np.arange(P)[:, None]
    d = np.arange(P)[None, :]
    msk = np.where(d <= r, 0.0, NEG).astype(np.float16)   # causal 128-block
    ident = np.eye(P, dtype=np.float16)

    in_maps = []
    for b in range(B):
        in_maps.append({
            "xt": np.ascontiguousarray(x[b].T).astype(np.float16),
            "wkqv": wkqv,
            "wo": wo,
            "msk": msk,
            "ident": ident,
        })
    return in_maps


def _run(x, W_K, W_Q, W_V, W_O, **spmd_kwargs):
    from concourse.bass_utils import run_bass_kernel_spmd

    if "nc" not in _cache:
        _cache["nc"] = _build_nc()
    in_maps = _host_prep(
        np.asarray(x, dtype=np.float32), np.asarray(W_K, dtype=np.float32),
        np.asarray(W_Q, dtype=np.float32), np.asarray(W_V, dtype=np.float32),
        np.asarray(W_O, dtype=np.float32),
    )
    res = run_bass_kernel_spmd(_cache["nc"], in_maps, core_ids=list(range(B)),
                               **spmd_kwargs)
    out = np.stack([r["out"] for r in res.results], axis=0).astype(np.float32)
    return out, res


def kernel(x, W_K, W_Q, W_V, W_O):
    out, _ = _run(x, W_K, W_Q, W_V, W_O)
    return out



# revision 3
# speedup vs baseline: 1.1551x; 1.1551x over previous
"""Trainium2 Bass kernel for 16-head causal attention (transposed-softmax variant).

Problem shapes: x [8, 1024, 1024]; W_K/W_Q/W_V [16, 64, 1024]; W_O [1024, 1024].
Sharding: pure data-parallel over batch (8 batch elements -> 8 cores), weights
replicated, no collectives.

v2 schedule: the scalar (ACT) engine is the scarce resource (~154us of exp +
accum-read per core vs ~160us of PE streaming), so attention is spread evenly
across the whole kernel instead of running after the projections. The 64
(pair, row) exp steps form the pacing backbone; all dense GEMM work (QKV
projections, AV, output projection) is emitted as PE "fillers" between row
steps, sized by a static credit model of estimated PE vs ACT time.

Per-core pipeline details:
  1. Inputs are host-packed into consumption-ordered DRAM slabs so each SBUF
     destination region is one dma_start (the sync-queue issue rate, ~0.6us
     per DMA, otherwise delays the first attention rows).
  2. Scores S[c,C] per (pair, row): two heads' K=64 matmuls adjacent in the
     PE queue -> concurrent in disjoint row groups; triangular mask of the
     diagonal 128-block accumulated via identity-matmul; exp on ACT with
     accum_out giving row sums (Q pre-scaled by 1/sqrt(d_head) on host).
  3. PSUM: 2x[128,1024] slots dedicated to long rows (i>=4) + 4x[128,512]
     shared by short rows / QKV / AV / output projection = all 8 banks.
  4. Normalization: one batched reciprocal per pair on [128,16] sums;
     V rows scaled in place during the next pair; AV accumulates ascending
     over c-tiles (per-element has_written handles the triangular growth),
     one 512-col PSUM bank at a time.
  5. Output projection is split 6+2: pairs 0-5 contributions (stage A)
     computed during pairs 6-7 and cast to fp16 partials; the tail only runs
     pair 6+7 matmuls and a fused PSUM+SBUF add per tile.
"""

import numpy as np

S, E, A, H, B = 1024, 1024, 16, 64, 8
P = 128          # partitions
NEG = -30000.0   # additive mask value (fp16-safe; exp -> 0 in fp32)
ET = E // P      # 8 e-tiles
CT = S // P      # 8 c-tiles
NC = S // 512    # 2 512-chunks
NPAIR = A // 2   # 8 head pairs

_cache = {}


def _off(i):
    """Compact E-buffer offset of row-tile i (valid width of row i is (i+1)*P)."""
    return P * i * (i + 1) // 2


EW = _off(8)     # 4608 columns total


def _build_nc():
    import concourse.bass as bass
    import concourse.mybir as mybir
    from concourse.tile import TileContext

    f16 = mybir.dt.float16
    f32 = mybir.dt.float32
    Exp = mybir.ActivationFunctionType.Exp
    Alu = mybir.AluOpType

    nc = bass.Bass()
    # consumption-ordered DRAM slabs (see _host_prep for layouts)
    msk_d = nc.dram_tensor("msk", [P, P], f16, kind="ExternalInput")
    id_d = nc.dram_tensor("ident", [P, P], f16, kind="ExternalInput")
    k01_d = nc.dram_tensor("k01", [P, ET, 2 * P], f16, kind="ExternalInput")
    q01_d = nc.dram_tensor("q01", [P, ET, 2 * P], f16, kind="ExternalInput")
    xq_d = nc.dram_tensor("xq", [P, 4, ET, 256], f16, kind="ExternalInput")
    vf_d = nc.dram_tensor("vf", [P, 2, ET, 512], f16, kind="ExternalInput")
    kqr_d = nc.dram_tensor("kqr", [P, 12, ET, P], f16, kind="ExternalInput")
    wo_d = nc.dram_tensor("wo", [P, ET, E], f16, kind="ExternalInput")
    out_d = nc.dram_tensor("out", [S, E], f16, kind="ExternalOutput")

    with TileContext(nc) as tc:
        with (
            tc.tile_pool(name="inp", bufs=1) as inp,
            tc.tile_pool(name="kqv", bufs=1) as kqv,
            tc.tile_pool(name="epool", bufs=4) as epool,
            tc.tile_pool(name="stats", bufs=3) as stats,
            tc.tile_pool(name="outp", bufs=4) as outp,
            tc.tile_pool(name="psq", bufs=4, space="PSUM") as psq,
            tc.tile_pool(name="scl", bufs=2, space="PSUM") as scl,
        ):
            # ---- SBUF destinations ----
            xT = inp.tile([P, ET, S], f16, tag="xT")
            wkqv = inp.tile([P, ET, 3 * A * H], f16, tag="wkqv")
            wo = inp.tile([P, ET, E], f16, tag="wo")
            msk = inp.tile([P, P], f16, tag="msk")
            ident = inp.tile([P, P], f16, tag="ident")

            K_T = kqv.tile([P, NPAIR, S], f16, tag="K_T")   # pair-stacked [2h, c]
            Q_T = kqv.tile([P, NPAIR, S], f16, tag="Q_T")
            V = kqv.tile([P, CT, A * H], f16, tag="V")       # [c, f]
            zT = kqv.tile([P, NPAIR, S], f16, tag="zT")      # pair-stacked [f, C]
            outA = kqv.tile([P, CT * NC, 512], f16, tag="outA")  # stage-A partials

            # ---- DMA: one issue per consumption-ordered slab ----
            nc.sync.dma_start(ident[:], id_d[:])
            nc.sync.dma_start(msk[:], msk_d[:])
            nc.sync.dma_start(wkqv[:, :, 0:2 * P], k01_d[:])
            nc.sync.dma_start(wkqv[:, :, A * H:A * H + 2 * P], q01_d[:])
            for q in range(4):
                nc.sync.dma_start(xT[:, :, q * 256:(q + 1) * 256], xq_d[:, q])
            nc.sync.dma_start(wkqv[:, :, 2 * A * H:2 * A * H + 512], vf_d[:, 0])
            for pi, p in enumerate(range(2, 8)):
                nc.sync.dma_start(wkqv[:, :, p * P:(p + 1) * P], kqr_d[:, 2 * pi])
                nc.sync.dma_start(wkqv[:, :, A * H + p * P:A * H + (p + 1) * P],
                                  kqr_d[:, 2 * pi + 1])
            nc.sync.dma_start(wkqv[:, :, 2 * A * H + 512:3 * A * H], vf_d[:, 1])
            nc.sync.dma_start(wo[:, :, :], wo_d[:])

            # ---- preload the exp table set while DMAs are in flight ----
            dmy = stats.tile([P, 1], f32, tag="dmy")
            dmy2 = stats.tile([P, 1], f32, tag="dmy2")
            nc.vector.memset(dmy[:], 0.0)
            nc.scalar.activation(dmy2[:], dmy[:], Exp)

            # ---- PE warm-up: ride out the HAM throttle during the DMA wait ----
            wps = psq.tile([P, 512], f32, tag="psq", name="warm")
            for w in range(38):
                nc.tensor.matmul(wps[:, :P], ident[:], msk[:],
                                 start=(w == 0), stop=(w == 37),
                                 skip_group_check=True)
            wsb = stats.tile([P, 1], f32, tag="wsink")
            nc.vector.reduce_max(wsb[:], wps[:, :P], axis=mybir.AxisListType.X)

            # ---- credit model state ----
            st = {"pe": 0.0, "act": 0.0}

            # ---- dense projection groups ----
            def kq_group(p, mat, cc):
                dst = K_T if mat == 0 else Q_T
                ps = psq.tile([P, 512], f32, tag="psq", name=f"q{p}{mat}{cc}")
                for et in range(ET):
                    nc.tensor.matmul(
                        ps[:],
                        wkqv[:, et, mat * A * H + p * P: mat * A * H + (p + 1) * P],
                        xT[:, et, cc * 512:(cc + 1) * 512],
                        start=(et == 0), stop=(et == ET - 1),
                    )
                nc.vector.tensor_copy(out=dst[:, p, cc * 512:(cc + 1) * 512], in_=ps[:])

            def v_group(fc, i):
                ps = psq.tile([P, 512], f32, tag="psq", name=f"v{fc}{i}")
                for et in range(ET):
                    nc.tensor.matmul(
                        ps[:],
                        xT[:, et, i * P:(i + 1) * P],
                        wkqv[:, et, 2 * A * H + fc * 512: 2 * A * H + (fc + 1) * 512],
                        start=(et == 0), stop=(et == ET - 1),
                    )
                nc.vector.tensor_copy(out=V[:, i, fc * 512:(fc + 1) * 512], in_=ps[:])

            Es = {}
            rcps = {}

            def av_a(p):
                """V-scale (batched rcp) + AV chunk j=0, ascending c-tiles."""
                Ets = Es[p]
                rcp_p = rcps[p]
                heads = [(2 * p, 0), (2 * p + 1, H)]
                for i in range(CT):
                    for k, (a, off) in enumerate(heads):
                        nc.vector.tensor_scalar_mul(
                            V[:, i, a * H:(a + 1) * H],
                            V[:, i, a * H:(a + 1) * H],
                            rcp_p[:, k, i:i + 1],
                        )
                za = psq.tile([P, 512], f32, tag="psq", name=f"za0_{p}")
                for i in range(CT):
                    ntrim = min(512, i * P + P)
                    for k, (a, off) in enumerate(heads):
                        nc.tensor.matmul(
                            za[off:off + H, :ntrim],
                            V[:, i, a * H:(a + 1) * H],
                            Ets[k][:, _off(i):_off(i) + ntrim],
                            start=(i == 0), stop=(i == CT - 1),
                            skip_group_check=True,
                        )
                nc.vector.tensor_copy(out=zT[:, p, 0:512], in_=za[:])

            def av_b(p):
                """AV chunk j=1 (C 512:1024), rows 4-7."""
                Ets = Es[p]
                heads = [(2 * p, 0), (2 * p + 1, H)]
                za = psq.tile([P, 512], f32, tag="psq", name=f"za1_{p}")
                for i in range(4, CT):
                    ntrim = min(512, (i - 4) * P + P)
                    for k, (a, off) in enumerate(heads):
                        nc.tensor.matmul(
                            za[off:off + H, :ntrim],
                            V[:, i, a * H:(a + 1) * H],
                            Ets[k][:, _off(i) + 512:_off(i) + 512 + ntrim],
                            start=(i == 4), stop=(i == CT - 1),
                            skip_group_check=True,
                        )
                nc.vector.tensor_copy(out=zT[:, p, 512:1024], in_=za[:])

            def stg_a(t):
                m, n_ = t // NC, t % NC
                ps = psq.tile([P, 512], f32, tag="psq", name=f"A{t}")
                for p2 in range(6):
                    nc.tensor.matmul(
                        ps[:],
                        zT[:, p2, m * P:(m + 1) * P],
                        wo[:, p2, n_ * 512:(n_ + 1) * 512],
                        start=(p2 == 0), stop=(p2 == 5),
                    )
                nc.vector.tensor_copy(out=outA[:, t, :], in_=ps[:])

            # ---- filler queue: (need, avail, cost_ns, fn) in priority order ----
            queue = []

            def q_kq(p, cc, need):
                for mat in (0, 1):
                    queue.append(dict(need=need, avail=0, cost=1780,
                                      fn=lambda p=p, mat=mat, cc=cc: kq_group(p, mat, cc)))

            q_kq(0, 1, 0.5)
            q_kq(1, 0, 1)
            q_kq(1, 1, 1.5)
            for i in range(CT):
                queue.append(dict(need=1.6, avail=0, cost=1780,
                                  fn=lambda i=i: v_group(0, i)))
            for p in range(2, 8):
                q_kq(p, 0, p)
                if p <= 6:   # AV(0)..AV(4); AV(5)/AV(6) placed below
                    queue.append(dict(need=p, avail=p - 1, cost=1700,
                                      fn=lambda p=p - 2: av_a(p)))
                    queue.append(dict(need=p, avail=p - 1, cost=750,
                                      fn=lambda p=p - 2: av_b(p)))
                q_kq(p, 1, p + 0.5)
                if p == 4:
                    for i in range(CT):
                        queue.append(dict(need=4.6, avail=0, cost=1780,
                                          fn=lambda i=i: v_group(1, i)))
            # pairs 6/7 span: AV(5), stage A first half, AV(6), stage A rest
            queue.append(dict(need=7, avail=6, cost=1700, fn=lambda: av_a(5)))
            queue.append(dict(need=7, avail=6, cost=750, fn=lambda: av_b(5)))
            for t in range(8):
                queue.append(dict(need=9, avail=6, cost=1300,
                                  fn=lambda t=t: stg_a(t)))
            queue.append(dict(need=8, avail=7, cost=1700, fn=lambda: av_a(6)))
            queue.append(dict(need=8, avail=7, cost=750, fn=lambda: av_b(6)))
            for t in range(8, 16):
                queue.append(dict(need=9, avail=7, cost=1300,
                                  fn=lambda t=t: stg_a(t)))

            def emit_head():
                item = queue.pop(0)
                item["fn"]()
                st["pe"] += item["cost"]

            def pump(cur):
                while queue and queue[0]["avail"] <= cur and \
                        st["pe"] + 0.5 * queue[0]["cost"] < st["act"] - 400.0:
                    emit_head()

            def force(key, cur):
                while queue and queue[0]["need"] <= key and queue[0]["avail"] <= cur:
                    emit_head()

            # ---- attention backbone ----
            def row_step(p, i, rows, Ets, sc_p):
                vw = (i + 1) * P
                n_i = i // 4 + 1
                heads = [(2 * p, 0), (2 * p + 1, H)]
                for j in range(n_i):
                    diag = j == n_i - 1
                    ntrim = (i * P - j * 512) + P if diag else 512
                    for k, (a, off) in enumerate(heads):
                        nc.tensor.matmul(
                            rows[k][:, j * 512:j * 512 + ntrim],
                            K_T[off:off + H, p, i * P:(i + 1) * P],
                            Q_T[off:off + H, p, j * 512:j * 512 + ntrim],
                            start=True, stop=not diag,
                            skip_group_check=True,
                        )
                for k in range(2):
                    nc.tensor.matmul(
                        rows[k][:, i * P:(i + 1) * P],
                        ident[:], msk[:],
                        start=False, stop=True,
                        skip_group_check=True,
                    )
                st["pe"] += vw / 2.4 + 250.0
                for k in range(2):
                    nc.scalar.activation(
                        Ets[k][:, _off(i):_off(i) + vw], rows[k][:, :vw], Exp,
                        accum_out=sc_p[:, k, i:i + 1],
                    )
                    st["act"] += (vw + 352.0) / 1.2 + 430.0

            # first two projection groups inline (nothing else for PE yet)
            kq_group(0, 0, 0)
            kq_group(0, 1, 0)
            st["pe"] += 2 * 1780

            for p in range(NPAIR):
                force(p, p)
                Ets = [epool.tile([P, EW], f16, tag="E", name=f"E{k}_{p}")
                       for k in range(2)]
                Es[p] = Ets
                sc_p = stats.tile([P, 2, 8], f32, tag="sc", name=f"sc{p}")
                for i in range(CT):
                    if i == 4:
                        force(p + 0.5, p)
                    if i < 4:
                        rows = [psq.tile([P, 512], f32, tag="psq", name=f"r{p}_{i}_{k}")
                                for k in range(2)]
                    else:
                        rows = [scl.tile([P, 1024], f32, tag="scl", name=f"r{p}_{i}_{k}")
                                for k in range(2)]
                    row_step(p, i, rows, Ets, sc_p)
                    pump(p)
                rcp_p = stats.tile([P, 2, 8], f32, tag="rcp", name=f"rcp{p}")
                nc.vector.reciprocal(rcp_p[:], sc_p[:])
                rcps[p] = rcp_p

            # ---- tail: drain queue, AV of pair 7, output projection stage B ----
            force(99, 99)
            av_a(7)
            av_b(7)
            for t in range(CT * NC):
                m, n_ = t // NC, t % NC
                ps = psq.tile([P, 512], f32, tag="psq", name=f"B{t}")
                for p2 in (6, 7):
                    nc.tensor.matmul(
                        ps[:],
                        zT[:, p2, m * P:(m + 1) * P],
                        wo[:, p2, n_ * 512:(n_ + 1) * 512],
                        start=(p2 == 6), stop=(p2 == 7),
                    )
                ot = outp.tile([P, 512], f16, tag="ot")
                nc.vector.scalar_tensor_tensor(
                    ot[:], ps[:], 1.0, outA[:, t, :], Alu.mult, Alu.add,
                )
                last = t == CT * NC - 1
                nq = 4 if last else 1
                w = 512 // nq
                for hh in range(nq):
                    nc.sync.dma_start(
                        out_d[m * P:(m + 1) * P,
                              n_ * 512 + hh * w:n_ * 512 + (hh + 1) * w],
                        ot[:, hh * w:(hh + 1) * w],
                    )

    # HW allows only one sync-wait per instruction (matmuls especially);
    # split excess waits into InstEventSemaphore like the bacc layer does.
    import bass_rust
    bass_rust.generate_event_semaphores(nc)
    return nc


def _host_prep(x, W_K, W_Q, W_V, W_O):
    """Pack per-core input dicts into consumption-ordered fp16 DRAM slabs."""
    wk = W_K.transpose(2, 0, 1).reshape(E, A * H)
    wq = (W_Q / np.sqrt(H)).transpose(2, 0, 1).reshape(E, A * H)
    wv = W_V.transpose(2, 0, 1).reshape(E, A * H)

    def slab(mat, c0, c1):
        # [E, cols] -> [P, ET, c1-c0]: partition p, e-tile t holds row t*P+p
        return np.ascontiguousarray(
            mat.reshape(ET, P, A * H)[:, :, c0:c1].transpose(1, 0, 2)
        ).astype(np.float16)

    k01 = slab(wk, 0, 2 * P)
    q01 = slab(wq, 0, 2 * P)
    vf = np.stack([slab(wv, 0, 512), slab(wv, 512, 1024)], axis=1)
    kqr = np.stack(
        [slab((wk, wq)[m], p * P, (p + 1) * P) for p in range(2, 8) for m in (0, 1)],
        axis=1,
    )
    wo_pk = np.ascontiguousarray(
        W_O.T.reshape(ET, P, E).transpose(1, 0, 2)
    ).astype(np.float16)

    r = np.arange(P)[:, None]
    d = np.arange(P)[None, :]
    msk = np.where(d <= r, 0.0, NEG).astype(np.float16)   # causal 128-block
    ident = np.eye(P, dtype=np.float16)

    in_maps = []
    for b in range(B):
        xt = np.ascontiguousarray(x[b].T).astype(np.float16)   # [E, S]
        xq = np.ascontiguousarray(
            xt.reshape(ET, P, 4, 256).transpose(1, 2, 0, 3)
        )  # [P, 4, ET, 256]
        in_maps.append({
            "xq": xq,
            "k01": k01,
            "q01": q01,
            "vf": vf,
            "kqr": kqr,
            "wo": wo_pk,
            "msk": msk,
            "ident": ident,
        })
    return in_maps


def _run(x, W_K, W_Q, W_V, W_O, **spmd_kwargs):
    from concourse.bass_utils import run_bass_kernel_spmd

    if "nc" not in _cache:
        _cache["nc"] = _build_nc()
    in_maps = _host_prep(
        np.asarray(x, dtype=np.float32), np.asarray(W_K, dtype=np.float32),
        np.asarray(W_Q, dtype=np.float32), np.asarray(W_V, dtype=np.float32),
        np.asarray(W_O, dtype=np.float32),
    )
    res = run_bass_kernel_spmd(_cache["nc"], in_maps, core_ids=list(range(B)),
                               **spmd_kwargs)
    out = np.stack([r["out"] for r in res.results], axis=0).astype(np.float32)
    return out, res


def kernel(x, W_K, W_Q, W_V, W_O):
    out, _ = _run(x, W_K, W_Q, W_V, W_O)
    return out
